# revision 7
# baseline (speedup 1.0000x reference)
"""Multi-head self-attention (B=4, S=2048, D=1024, H=16, Hd=64) on 8 TRN2 cores.

Sharding: core c -> (batch b = c//2, head-group g = c%2 of 8 heads).
Each core computes its batch's 8 heads end-to-end plus the partial output
projection for its head group; the host sums the two head-group partials
per batch. No collectives.

Device layout is fully transposed: activations are [feature(partitions),
seq(free)], so the matmul chain QKV -> scores -> PV -> out-proj needs no
on-device transposes, and the softmax k-sum is computed by the PE via an
all-ones column appended to each head's V slice (denominator lands in
PSUM partition 64 of the PV output).
"""

from contextlib import ExitStack

import numpy as np
import ml_dtypes

import concourse.bass as bass
import concourse.tile as tile
from concourse import mybir
from concourse.bass_utils import run_bass_kernel_spmd
from concourse.vector_clock import ScopedClock
from bass_rust import InstNoOp, SyncInfo

BF16 = mybir.dt.bfloat16
F32 = mybir.dt.float32
AF = mybir.ActivationFunctionType

B, S, D = 4, 2048, 1024
H, HD = 16, 64
GH = 8          # heads per core (head-group size)
GM = GH * HD    # 512 head dims per core
NQB = 4         # q blocks of 512
QB = 512
NKC = 16        # k chunks of 128
NDC = 8         # d chunks of 128 (contraction for projections)

_META_TYPES = ("TileBranchInst", "BassTileLoopBlock", "BassTilePoolBoundary")


class _TileCtx(tile.TileContext):
    """Splits multi-sem-wait instructions: the pinned walrus rejects any TPB
    instruction carrying more than one sem-wait, while Tile emits joins and a
    global end-of-context drain with several."""

    def _split_waits(self, ordered):
        nc = self.nc
        for bb_name, insts in ordered.items():
            out = []
            for inst in insts:
                si = inst.sync_info
                if (
                    si is not None
                    and si.on_wait
                    and len(si.on_wait) > 1
                    and type(inst).__name__ not in _META_TYPES
                    and inst.engine != mybir.EngineType.Unassigned
                ):
                    waits = list(si.on_wait)
                    for w in waits[:-1]:
                        nop = InstNoOp(
                            name=nc.get_next_instruction_name(), ins=[], outs=[]
                        )
                        nop.engine = inst.engine
                        nop.sync_info = SyncInfo(on_wait=[w], on_update=[])
                        out.append(nop)
                    inst.sync_info = SyncInfo(
                        on_wait=[waits[-1]], on_update=list(si.on_update)
                    )
                out.append(inst)
            ordered[bb_name] = out

    def _lower_ordered_insts(self, ordered):
        self._split_waits(ordered)
        super()._lower_ordered_insts(ordered)

    def _drain_and_barrier(self, tick_clock, wait_clock):
        drain_inst = self.nc.sync.drain()
        wait_clock.add_sem_waits(
            drain_inst.ins, ScopedClock({None: tick_clock.global_clock})
        )
        si = drain_inst.ins.sync_info
        waits = list(si.on_wait) if si is not None else []
        if len(waits) > 1:
            drain_inst.ins.sync_info = SyncInfo(
                on_wait=waits[:1], on_update=list(si.on_update)
            )
            for w in waits[1:]:
                extra = self.nc.sync.drain()
                extra.ins.sync_info = SyncInfo(on_wait=[w], on_update=[])

        self.nc.all_engine_barrier()
        assert self.sems is not None
        popped = self.nc._tile_sem_poison_stack.pop()
        assert popped is self._sem_poison
        self.nc.clear_and_free_semaphores(list(self.sems.allocated().values()))
        self.nc.all_engine_barrier()


def _build_program():
    nc = bass.Bass(trn_type="TRN2", debug=False, num_devices=8)

    xT = nc.dram_tensor("xT", [D, S], BF16, kind="ExternalInput").ap()
    wq = nc.dram_tensor("wq", [D, GM], BF16, kind="ExternalInput").ap()
    wk = nc.dram_tensor("wk", [D, GM], BF16, kind="ExternalInput").ap()
    wv = nc.dram_tensor("wv", [D, GM], BF16, kind="ExternalInput").ap()
    # per-head-reordered Wo.T slice: [64, 8 heads x 1024]
    wo = nc.dram_tensor("wo", [HD, GH * D], BF16, kind="ExternalInput").ap()
    bq = nc.dram_tensor("bq", [GM], F32, kind="ExternalInput").ap()
    bk = nc.dram_tensor("bk", [GM], F32, kind="ExternalInput").ap()
    bo = nc.dram_tensor("bo", [D], F32, kind="ExternalInput").ap()
    outT = nc.dram_tensor("outT", [D, S], F32, kind="ExternalOutput").ap()

    with _TileCtx(nc) as tc, ExitStack() as ctx:
        const_pool = ctx.enter_context(tc.tile_pool(name="const", bufs=1))
        w_pool = ctx.enter_context(tc.tile_pool(name="wts", bufs=1))
        act_pool = ctx.enter_context(tc.tile_pool(name="acts", bufs=1))

        # ---- constants / weights / inputs -------------------------------
        ones = const_pool.tile([128, 128], F32, tag="ones")
        nc.vector.memset(ones[:], 1.0)
        bq_sb = const_pool.tile([128, NDC // 2], F32, tag="bq")
        nc.sync.dma_start(bq_sb[:], bq.rearrange("(c p) -> p c", p=128))
        bk_sb = const_pool.tile([128, NDC // 2], F32, tag="bk")
        nc.sync.dma_start(bk_sb[:], bk.rearrange("(c p) -> p c", p=128))
        bo_sb = const_pool.tile([128, NDC], F32, tag="bo")
        nc.sync.dma_start(bo_sb[:], bo.rearrange("(c p) -> p c", p=128))

        xt = act_pool.tile([128, NDC * S], BF16, tag="xt")
        for t in range(NDC):
            nc.sync.dma_start(
                xt[:, t * S : (t + 1) * S], xT[t * 128 : (t + 1) * 128, :]
            )
        wq_sb = w_pool.tile([128, NDC * GM], BF16, tag="wq")
        nc.sync.dma_start(
            wq_sb[:].rearrange("p (c m) -> p c m", m=GM),
            wq.rearrange("(c p) m -> p c m", p=128),
        )
        wk_sb = w_pool.tile([128, NDC * GM], BF16, tag="wk")
        nc.sync.dma_start(
            wk_sb[:].rearrange("p (c m) -> p c m", m=GM),
            wk.rearrange("(c p) m -> p c m", p=128),
        )
        wv_sb = w_pool.tile([128, NDC * GM], BF16, tag="wv")
        nc.sync.dma_start(
            wv_sb[:].rearrange("p (c m) -> p c m", m=GM),
            wv.rearrange("(c p) m -> p c m", p=128),
        )
        wo_sb = w_pool.tile([HD, GH * D], BF16, tag="wo")
        nc.sync.dma_start(wo_sb[:], wo[:, :])

        # ---- QKV projections --------------------------------------------
        # Q.T, K.T: [GM(m, partition-tiled), S(q)]
        qt = act_pool.tile([128, (GM // 128) * S], BF16, tag="qt")
        kt = act_pool.tile([128, (GM // 128) * S], BF16, tag="kt")
        # V (not transposed): [S(k, partition-tiled), 8 heads x 65] with an
        # all-ones column after each head's 64 dims.
        VW = GH * (HD + 1)  # 520
        v_sb = act_pool.tile([128, NKC * VW], BF16, tag="v")
        nc.vector.memset(v_sb[:], 1.0)

        with tc.tile_pool(name="qk_psum", bufs=2, space="PSUM") as qk_psum:
            for w_sb, b_sb, dst in ((wq_sb, bq_sb, qt), (wk_sb, bk_sb, kt)):
                for mi in range(GM // 128):
                    ps = qk_psum.tile([128, S], F32, tag="qkp")
                    for dc in range(NDC):
                        lhsT = w_sb[:, dc * GM + mi * 128 : dc * GM + (mi + 1) * 128]
                        for qb in range(NQB):
                            nc.tensor.matmul(
                                ps[:, qb * QB : (qb + 1) * QB],
                                lhsT,
                                xt[:, dc * S + qb * QB : dc * S + (qb + 1) * QB],
                                start=(dc == 0),
                                stop=(dc == NDC - 1),
                            )
                    nc.vector.tensor_scalar_add(
                        dst[:, mi * S : (mi + 1) * S], ps[:], b_sb[:, mi : mi + 1]
                    )
        with tc.tile_pool(name="v_psum", bufs=4, space="PSUM") as v_psum:
            for si in range(NKC):
                ps = v_psum.tile([128, GM], F32, tag="vp")
                for dc in range(NDC):
                    nc.tensor.matmul(
                        ps[:],
                        xt[:, dc * S + si * 128 : dc * S + (si + 1) * 128],
                        wv_sb[:, dc * GM : (dc + 1) * GM],
                        start=(dc == 0),
                        stop=(dc == NDC - 1),
                    )
                nc.vector.tensor_copy(
                    v_sb[:, si * VW : (si + 1) * VW]
                    .rearrange("p (h m) -> p h m", h=GH)[:, :, 0:HD],
                    ps[:].rearrange("p (h m) -> p h m", h=GH),
                )

        # ---- attention ---------------------------------------------------
        # O.T per head: [64, S], all at partition base 0.
        ot = [act_pool.tile([HD, S], BF16, name=f"ot{h}", tag=f"ot{h}") for h in range(GH)]

        with tc.tile_pool(name="s_psum", bufs=1, space="PSUM") as s_psum, \
             tc.tile_pool(name="o_psum", bufs=2, space="PSUM") as o_psum, \
             tc.tile_pool(name="b_psum", bufs=2, space="PSUM") as b_psum, \
             tc.tile_pool(name="slab", bufs=6) as slab_pool, \
             tc.tile_pool(name="norm", bufs=3) as norm_pool:
            for h in range(GH):
                mt = h // 2          # m-tile holding this head in qt/kt
                pb = (h % 2) * 64    # partition base of this head's rows
                for qb in range(NQB):
                    q_rhs = qt[pb : pb + 64, mt * S + qb * QB : mt * S + (qb + 1) * QB]
                    slabs = []
                    for g in range(4):  # 4 groups of 4 k-chunks
                        ps = s_psum.tile([128, 4 * QB], F32, tag="sp")
                        for j in range(4):
                            kc = g * 4 + j
                            lhsT = kt[
                                pb : pb + 64,
                                mt * S + kc * 128 : mt * S + (kc + 1) * 128,
                            ]
                            nc.tensor.matmul(
                                ps[:, j * QB : (j + 1) * QB],
                                lhsT,
                                q_rhs,
                                start=True,
                                stop=True,
                            )
                        sl = slab_pool.tile([128, 4 * QB], BF16, tag="slab")
                        nc.scalar.activation(sl[:], ps[:], AF.Exp, scale=0.125)
                        slabs.append(sl)
                    po = o_psum.tile([128, QB], F32, tag="op")
                    for kc in range(NKC):
                        nc.tensor.matmul(
                            po[0 : HD + 1, :],
                            v_sb[:, kc * VW + h * (HD + 1) : kc * VW + (h + 1) * (HD + 1)],
                            slabs[kc // 4][:, (kc % 4) * QB : (kc % 4 + 1) * QB],
                            start=(kc == 0),
                            stop=(kc == NKC - 1),
                        )
                    # denominator is row 64 of po; reciprocal + PE broadcast
                    recip = norm_pool.tile([128, QB], F32, tag="recip")
                    nc.vector.reciprocal(recip[64:65, :], po[64:65, :])
                    pb_ps = b_psum.tile([HD, QB], F32, tag="bp")
                    nc.tensor.matmul(
                        pb_ps[:],
                        ones[64:65, 0:HD],
                        recip[64:65, :],
                        start=True,
                        stop=True,
                    )
                    bcast = norm_pool.tile([HD, QB], F32, tag="bcast")
                    nc.vector.tensor_copy(bcast[:], pb_ps[:])
                    nc.vector.tensor_mul(
                        ot[h][:, qb * QB : (qb + 1) * QB], po[0:HD, :], bcast[:]
                    )

        # ---- output projection ------------------------------------------
        with tc.tile_pool(name="out_psum", bufs=2, space="PSUM") as out_psum, \
             tc.tile_pool(name="y", bufs=2) as y_pool:
            for ec in range(NDC):
                ps = out_psum.tile([128, S], F32, tag="yp")
                for h in range(GH):
                    lhsT = wo_sb[:, h * D + ec * 128 : h * D + (ec + 1) * 128]
                    for qb in range(NQB):
                        nc.tensor.matmul(
                            ps[:, qb * QB : (qb + 1) * QB],
                            lhsT,
                            ot[h][:, qb * QB : (qb + 1) * QB],
                            start=(h == 0),
                            stop=(h == GH - 1),
                        )
                y_sb = y_pool.tile([128, S], F32, tag="y")
                nc.vector.tensor_scalar_add(y_sb[:], ps[:], bo_sb[:, ec : ec + 1])
                nc.sync.dma_start(outT[ec * 128 : (ec + 1) * 128, :], y_sb[:])

    return nc


_NC = None
_last_in_maps = None


def _get_program():
    global _NC
    if _NC is None:
        _NC = _build_program()
    return _NC


def kernel(x, Wq, bq, Wk, bk, Wv, bv, Wo, bo):
    x = np.asarray(x, np.float32)
    bf = ml_dtypes.bfloat16
    in_maps = []
    for c in range(8):
        b, g = c // 2, c % 2
        sl = slice(g * GM, (g + 1) * GM)
        wo_slice = np.asarray(Wo, np.float32)[:, sl].T  # [512, 1024]
        # fold bv and half of bo into the output bias
        bo_eff = np.asarray(bo, np.float32) / 2.0 + np.asarray(bv, np.float32)[sl] @ wo_slice
        in_maps.append(
            {
                "xT": np.ascontiguousarray(x[b].T).astype(bf),
                "wq": np.ascontiguousarray(np.asarray(Wq, np.float32)[sl, :].T).astype(bf),
                "wk": np.ascontiguousarray(np.asarray(Wk, np.float32)[sl, :].T).astype(bf),
                "wv": np.ascontiguousarray(np.asarray(Wv, np.float32)[sl, :].T).astype(bf),
                "wo": np.ascontiguousarray(
                    wo_slice.reshape(GH, HD, D).transpose(1, 0, 2).reshape(HD, GH * D)
                ).astype(bf),
                "bq": np.ascontiguousarray(np.asarray(bq, np.float32)[sl]),
                "bk": np.ascontiguousarray(np.asarray(bk, np.float32)[sl]),
                "bo": np.ascontiguousarray(bo_eff.astype(np.float32)),
            }
        )

    global _last_in_maps
    _last_in_maps = in_maps
    nc = _get_program()
    res = run_bass_kernel_spmd(nc, in_maps, core_ids=list(range(8)))
    out = np.empty((B, S, D), np.float32)
    for b in range(B):
        acc = res.results[2 * b]["outT"].astype(np.float32) + res.results[
            2 * b + 1
        ]["outT"].astype(np.float32)
        out[b] = acc.T
    return out


# revision 10
# speedup vs baseline: 1.3776x; 1.3776x over previous
"""Multi-head self-attention (B=4, S=2048, D=1024, H=16, Hd=64) on 8 TRN2 cores.

Sharding: core c -> (batch b = c//2, head-group g = c%2 of 8 heads).
Each core computes its batch's 8 heads end-to-end plus the partial output
projection for its head group; the host sums the two head-group partials
per batch. No collectives.

Device layout is fully transposed: activations are [feature(partitions),
seq(free)], so the matmul chain QKV -> scores -> PV -> out-proj needs no
on-device transposes, and the softmax k-sum is computed by the PE via an
all-ones column appended to each head's V slice (denominator lands in
PSUM partition 64 of the PV output).
"""

from contextlib import ExitStack

import numpy as np
import ml_dtypes

import concourse.bass as bass
import concourse.tile as tile
from concourse import mybir
from concourse.bass_utils import run_bass_kernel_spmd
from concourse.vector_clock import ScopedClock
from bass_rust import InstNoOp, SyncInfo

BF16 = mybir.dt.bfloat16
F32 = mybir.dt.float32
F32R = mybir.dt.float32r
AF = mybir.ActivationFunctionType

B, S, D = 4, 2048, 1024
H, HD = 16, 64
GH = 8          # heads per core (head-group size)
GM = GH * HD    # 512 head dims per core
NQB = 4         # q blocks of 512
QB = 512
NKC = 16        # k chunks of 128
NDC = 8         # d chunks of 128 (contraction for projections)

_META_TYPES = ("TileBranchInst", "BassTileLoopBlock", "BassTilePoolBoundary")


class _TileCtx(tile.TileContext):
    """Splits multi-sem-wait instructions: the pinned walrus rejects any TPB
    instruction carrying more than one sem-wait, while Tile emits joins and a
    global end-of-context drain with several."""

    def _split_waits(self, ordered):
        nc = self.nc
        for bb_name, insts in ordered.items():
            out = []
            for inst in insts:
                si = inst.sync_info
                if (
                    si is not None
                    and si.on_wait
                    and len(si.on_wait) > 1
                    and type(inst).__name__ not in _META_TYPES
                    and inst.engine != mybir.EngineType.Unassigned
                ):
                    waits = list(si.on_wait)
                    for w in waits[:-1]:
                        nop = InstNoOp(
                            name=nc.get_next_instruction_name(), ins=[], outs=[]
                        )
                        nop.engine = inst.engine
                        nop.sync_info = SyncInfo(on_wait=[w], on_update=[])
                        out.append(nop)
                    inst.sync_info = SyncInfo(
                        on_wait=[waits[-1]], on_update=list(si.on_update)
                    )
                out.append(inst)
            ordered[bb_name] = out

    def _lower_ordered_insts(self, ordered):
        self._split_waits(ordered)
        super()._lower_ordered_insts(ordered)

    def _drain_and_barrier(self, tick_clock, wait_clock):
        drain_inst = self.nc.sync.drain()
        wait_clock.add_sem_waits(
            drain_inst.ins, ScopedClock({None: tick_clock.global_clock})
        )
        si = drain_inst.ins.sync_info
        waits = list(si.on_wait) if si is not None else []
        if len(waits) > 1:
            drain_inst.ins.sync_info = SyncInfo(
                on_wait=waits[:1], on_update=list(si.on_update)
            )
            for w in waits[1:]:
                extra = self.nc.sync.drain()
                extra.ins.sync_info = SyncInfo(on_wait=[w], on_update=[])

        self.nc.all_engine_barrier()
        assert self.sems is not None
        popped = self.nc._tile_sem_poison_stack.pop()
        assert popped is self._sem_poison
        self.nc.clear_and_free_semaphores(list(self.sems.allocated().values()))
        self.nc.all_engine_barrier()


def _build_program():
    nc = bass.Bass(trn_type="TRN2", debug=False, num_devices=8)

    xT = nc.dram_tensor("xT", [D, S], BF16, kind="ExternalInput").ap()
    wq = nc.dram_tensor("wq", [D, GM], BF16, kind="ExternalInput").ap()
    wk = nc.dram_tensor("wk", [D, GM], BF16, kind="ExternalInput").ap()
    wv = nc.dram_tensor("wv", [D, GM], BF16, kind="ExternalInput").ap()
    # pair-major-reordered Wo.T slice: [128, 4 pairs x 1024]
    wo = nc.dram_tensor("wo", [128, (GM // 128) * D], BF16, kind="ExternalInput").ap()
    bq = nc.dram_tensor("bq", [GM], F32, kind="ExternalInput").ap()
    bk = nc.dram_tensor("bk", [GM], F32, kind="ExternalInput").ap()
    bo = nc.dram_tensor("bo", [D], F32, kind="ExternalInput").ap()
    outT = nc.dram_tensor("outT", [D, S], F32, kind="ExternalOutput").ap()

    with _TileCtx(nc) as tc, ExitStack() as ctx:
        const_pool = ctx.enter_context(tc.tile_pool(name="const", bufs=1))
        act_pool = ctx.enter_context(tc.tile_pool(name="acts", bufs=1))

        # ---- constants / weights / inputs -------------------------------
        ones = const_pool.tile([128, 128], F32, tag="ones")
        nc.vector.memset(ones[:], 1.0)
        bq_sb = const_pool.tile([128, NDC // 2], F32, tag="bq")
        nc.sync.dma_start(bq_sb[:], bq.rearrange("(c p) -> p c", p=128))
        bk_sb = const_pool.tile([128, NDC // 2], F32, tag="bk")
        nc.sync.dma_start(bk_sb[:], bk.rearrange("(c p) -> p c", p=128))
        bo_sb = const_pool.tile([128, NDC], F32, tag="bo")
        nc.sync.dma_start(bo_sb[:], bo.rearrange("(c p) -> p c", p=128))

        wo_sb = const_pool.tile([128, (GM // 128) * D], BF16, tag="wo")
        nc.sync.dma_start(wo_sb[:], wo[:, :])
        ones_r = const_pool.tile([128, HD], F32R, tag="ones_r")
        with nc.allow_low_precision(reason="f32r all-ones constant (exact)"):
            nc.vector.tensor_copy(ones_r[:], ones[:, 0:HD])

        phase1 = ExitStack()
        w_pool = phase1.enter_context(tc.tile_pool(name="wts", bufs=1))
        xt = w_pool.tile([128, NDC * S], BF16, tag="xt")
        for t in range(NDC):
            nc.sync.dma_start(
                xt[:, t * S : (t + 1) * S], xT[t * 128 : (t + 1) * 128, :]
            )
        wq_sb = w_pool.tile([128, NDC * GM], BF16, tag="wq")
        nc.sync.dma_start(
            wq_sb[:].rearrange("p (c m) -> p c m", m=GM),
            wq.rearrange("(c p) m -> p c m", p=128),
        )
        wk_sb = w_pool.tile([128, NDC * GM], BF16, tag="wk")
        nc.sync.dma_start(
            wk_sb[:].rearrange("p (c m) -> p c m", m=GM),
            wk.rearrange("(c p) m -> p c m", p=128),
        )
        wv_sb = w_pool.tile([128, NDC * GM], BF16, tag="wv")
        nc.sync.dma_start(
            wv_sb[:].rearrange("p (c m) -> p c m", m=GM),
            wv.rearrange("(c p) m -> p c m", p=128),
        )
        # ---- QKV projections --------------------------------------------
        # Q.T, K.T: [GM(m, partition-tiled), S(q)]
        qt = act_pool.tile([128, (GM // 128) * S], BF16, tag="qt")
        kt = act_pool.tile([128, (GM // 128) * S], BF16, tag="kt")
        # V (not transposed): [S(k, partition-tiled), 8 heads x 65] with an
        # all-ones column after each head's 64 dims.
        VW = GH * (HD + 1)  # 520
        v_sb = act_pool.tile([128, NKC * VW], BF16, tag="v")
        nc.vector.memset(v_sb[:], 1.0)

        with tc.tile_pool(name="qk_psum", bufs=2, space="PSUM") as qk_psum:
            for w_sb, b_sb, dst in ((wq_sb, bq_sb, qt), (wk_sb, bk_sb, kt)):
                for mi in range(GM // 128):
                    ps = qk_psum.tile([128, S], F32, tag="qkp")
                    for dc in range(NDC):
                        lhsT = w_sb[:, dc * GM + mi * 128 : dc * GM + (mi + 1) * 128]
                        for qb in range(NQB):
                            nc.tensor.matmul(
                                ps[:, qb * QB : (qb + 1) * QB],
                                lhsT,
                                xt[:, dc * S + qb * QB : dc * S + (qb + 1) * QB],
                                start=(dc == 0),
                                stop=(dc == NDC - 1),
                            )
                    nc.vector.tensor_scalar_add(
                        dst[:, mi * S : (mi + 1) * S], ps[:], b_sb[:, mi : mi + 1]
                    )
        with tc.tile_pool(name="v_psum", bufs=4, space="PSUM") as v_psum:
            for si in range(NKC):
                ps = v_psum.tile([128, GM], F32, tag="vp")
                for dc in range(NDC):
                    nc.tensor.matmul(
                        ps[:],
                        xt[:, dc * S + si * 128 : dc * S + (si + 1) * 128],
                        wv_sb[:, dc * GM : (dc + 1) * GM],
                        start=(dc == 0),
                        stop=(dc == NDC - 1),
                    )
                nc.vector.tensor_copy(
                    v_sb[:, si * VW : (si + 1) * VW]
                    .rearrange("p (h m) -> p h m", h=GH)[:, :, 0:HD],
                    ps[:].rearrange("p (h m) -> p h m", h=GH),
                )

        phase1.close()

        # ---- attention ---------------------------------------------------
        # O.T per head-pair: [128, S]; even head rows 0-63, odd head 64-127.
        otp = [
            act_pool.tile([128, S], BF16, name=f"otp{t}", tag=f"otp{t}")
            for t in range(GH // 2)
        ]

        with tc.tile_pool(name="s_psum", bufs=1, space="PSUM") as s_psum, \
             tc.tile_pool(name="o_psum", bufs=2, space="PSUM") as o_psum, \
             tc.tile_pool(name="b_psum", bufs=2, space="PSUM") as b_psum, \
             tc.tile_pool(name="slab", bufs=10) as slab_pool, \
             tc.tile_pool(name="norm", bufs=3) as norm_pool, \
             tc.tile_pool(name="shift", bufs=2) as shift_pool:
            for t in range(GH // 2):  # head pairs (2t, 2t+1)
                for qb in range(NQB):
                    q0 = qt[0:64, t * S + qb * QB : t * S + (qb + 1) * QB]
                    q1 = qt[64:128, t * S + qb * QB : t * S + (qb + 1) * QB]
                    slabs = []
                    for j in range(8):  # k-chunk pairs
                        ps = s_psum.tile([128, 4 * QB], F32, tag="sp")
                        for i in range(2):
                            kc = 2 * j + i
                            ksl = slice(t * S + kc * 128, t * S + (kc + 1) * 128)
                            nc.tensor.matmul(
                                ps[:, i * 1024 : i * 1024 + 512],
                                kt[0:64, ksl], q0, start=True, stop=True,
                            )
                            nc.tensor.matmul(
                                ps[:, i * 1024 + 512 : i * 1024 + 1024],
                                kt[64:128, ksl], q1, start=True, stop=True,
                            )
                        sl = slab_pool.tile([128, 4 * QB], BF16, tag="slab")
                        nc.scalar.activation(sl[:], ps[:], AF.Exp, scale=0.125)
                        slabs.append(sl)
                    for i in range(2):  # heads within the pair
                        h = 2 * t + i
                        po = o_psum.tile([128, QB], F32, tag="op")
                        for kc in range(NKC):
                            rhs = slabs[kc // 2][
                                :, (kc % 2) * 1024 + i * 512 : (kc % 2) * 1024 + (i + 1) * 512
                            ]
                            nc.tensor.matmul(
                                po[0 : HD + 1, :],
                                v_sb[:, kc * VW + h * (HD + 1) : kc * VW + (h + 1) * (HD + 1)],
                                rhs,
                                start=(kc == 0),
                                stop=(kc == NKC - 1),
                            )
                        recip = norm_pool.tile([128, QB], F32R, tag="recip")
                        with nc.allow_low_precision(
                            reason="softmax denom reciprocal stored f32r for full-rate PE broadcast"
                        ):
                            nc.vector.reciprocal(recip[64:65, :], po[64:65, :])
                        pb_ps = b_psum.tile([HD, QB], F32, tag="bp")
                        nc.tensor.matmul(
                            pb_ps[:], ones_r[64:65, 0:HD], recip[64:65, :],
                            start=True, stop=True,
                        )
                        bcast = norm_pool.tile([HD, QB], F32, tag="bcast")
                        nc.vector.tensor_copy(bcast[:], pb_ps[:])
                        if i == 0:
                            nc.vector.tensor_mul(
                                otp[t][0:HD, qb * QB : (qb + 1) * QB], po[0:HD, :], bcast[:]
                            )
                        else:
                            tmp = shift_pool.tile([HD, QB], BF16, tag="tmp")
                            nc.vector.tensor_mul(tmp[:], po[0:HD, :], bcast[:])
                            nc.sync.dma_start(
                                otp[t][HD:128, qb * QB : (qb + 1) * QB], tmp[:]
                            )

        # ---- output projection ------------------------------------------
        with tc.tile_pool(name="out_psum", bufs=2, space="PSUM") as out_psum, \
             tc.tile_pool(name="y", bufs=2) as y_pool:
            for ec in range(NDC):
                ps = out_psum.tile([128, S], F32, tag="yp")
                for mt in range(GM // 128):
                    lhsT = wo_sb[:, mt * D + ec * 128 : mt * D + (ec + 1) * 128]
                    for qb in range(NQB):
                        nc.tensor.matmul(
                            ps[:, qb * QB : (qb + 1) * QB],
                            lhsT,
                            otp[mt][:, qb * QB : (qb + 1) * QB],
                            start=(mt == 0),
                            stop=(mt == GM // 128 - 1),
                        )
                y_sb = y_pool.tile([128, S], F32, tag="y")
                nc.vector.tensor_scalar_add(y_sb[:], ps[:], bo_sb[:, ec : ec + 1])
                nc.sync.dma_start(outT[ec * 128 : (ec + 1) * 128, :], y_sb[:])

    return nc


_NC = None
_last_in_maps = None


def _get_program():
    global _NC
    if _NC is None:
        _NC = _build_program()
    return _NC


def kernel(x, Wq, bq, Wk, bk, Wv, bv, Wo, bo):
    x = np.asarray(x, np.float32)
    bf = ml_dtypes.bfloat16
    in_maps = []
    for c in range(8):
        b, g = c // 2, c % 2
        sl = slice(g * GM, (g + 1) * GM)
        wo_slice = np.asarray(Wo, np.float32)[:, sl].T  # [512, 1024]
        # fold bv and half of bo into the output bias
        bo_eff = np.asarray(bo, np.float32) / 2.0 + np.asarray(bv, np.float32)[sl] @ wo_slice
        in_maps.append(
            {
                "xT": np.ascontiguousarray(x[b].T).astype(bf),
                "wq": np.ascontiguousarray(np.asarray(Wq, np.float32)[sl, :].T).astype(bf),
                "wk": np.ascontiguousarray(np.asarray(Wk, np.float32)[sl, :].T).astype(bf),
                "wv": np.ascontiguousarray(np.asarray(Wv, np.float32)[sl, :].T).astype(bf),
                "wo": np.ascontiguousarray(
                    wo_slice.reshape(GM // 128, 128, D).transpose(1, 0, 2).reshape(128, (GM // 128) * D)
                ).astype(bf),
                "bq": np.ascontiguousarray(np.asarray(bq, np.float32)[sl]),
                "bk": np.ascontiguousarray(np.asarray(bk, np.float32)[sl]),
                "bo": np.ascontiguousarray(bo_eff.astype(np.float32)),
            }
        )

    global _last_in_maps
    _last_in_maps = in_maps
    nc = _get_program()
    res = run_bass_kernel_spmd(nc, in_maps, core_ids=list(range(8)))
    out = np.empty((B, S, D), np.float32)
    for b in range(B):
        acc = res.results[2 * b]["outT"].astype(np.float32) + res.results[
            2 * b + 1
        ]["outT"].astype(np.float32)
        out[b] = acc.T
    return out
